# revision 42
# baseline (speedup 1.0000x reference)
"""Trainium2 Bass kernel for a single causal attention head.

Problem: x:(8,2048,1024) f32, per-head projections wq/wk/wv:(64,1024),
biases (64,). Output: softmax(causal(q k^T / sqrt(64))) @ v : (8,2048,64).

Strategy:
  - Data-parallel: batch b -> core b (8 cores, 1 batch each).
  - Host prep packs every input into partition-major, fully contiguous
    per-partition lines so each DMA is ~128 large descriptors:
      * xp:(P, NCH*DT*CH) fp16 - x[b] chunk-major/d-major per partition
        (8KB contiguous per partition per chunk).
      * wall:(P, DT*(P+HD)) fp16 - [wq*s|wk] and wv interleaved per d-tile.
      * bb:(P, 2) f32 - [bq*s;bk] and [bv;bv] columns.
  - Device (per core):
      * qk1 = [wq|wk]^T.T @ x: rows 0-63 = Q^T, rows 64-127 = K^T (PSUM
        accumulate over 8 d-tiles, fp16 matmuls, N=512 chunks).
      * qk2 = half-swapped copy of qk1 -> both Q^T and K^T live on both
        partition halves; scores for two k-tiles share the PE array via
        row packing.
      * vT (64,T) fp16, transposed back to (T,64) tiles via fp16 PE
        transpose, augmented with a ones column (softmax denominator
        rides along the PV matmul).
      * S^T = K^T.T @ Q^T per k-tile; P^T = exp(S^T) on ACT; causal mask
        via gpsimd affine_select restricted to the 128-col diagonal band.
      * Diagonal pairs run FIRST per chunk with column-trimmed scores/
        exp/mask/PV (fully-masked columns never computed); non-diagonal
        pairs follow full-range.
      * O^T_aug[65, T] accumulated in PSUM over k-tiles; row 64 = sum_j P^T.
      * attention for chunk ci emitted right after projection chunk ci.
  - Host post: out[b] = (O^T[0:64] / O^T[64:65]).T  (softmax normalization).
"""

import numpy as np

B, T, D, HD = 8, 2048, 1024, 64
P = 128          # SBUF partitions
CH = 512         # q-chunk (matmul moving dim)
NCH = T // CH    # 4
DT = D // P      # 8 d-tiles
NKT = T // P     # 16 k-tiles
NWARM = 13       # PE clock-ramp warmup matmuls
DH = DT // 2     # d-tiles per combined/half x load (two DMA queues)

LAST_RESULTS = None      # BassKernelResults of the most recent run (for test.py)


def _build_module(legalize=True):
    import concourse.bass as bass
    import concourse.mybir as mybir
    from concourse.tile import TileContext

    from concourse.masks import make_identity
    F32 = mybir.dt.float32
    F16 = mybir.dt.float16

    nc = bass.Bass("TRN2", target_bir_lowering=True)

    WXC = DH * (P + CH)  # cols of a combined [w1-half | x0-half] tensor
    WXA = WXC + 4            # wxa also carries the biases (4 f16 = 2 f32)
    WXB = WXC + DT * HD      # wxb also carries wv
    xp = nc.dram_tensor("xp", (P, NCH * DT * CH), F16, kind="ExternalInput")
    wxa = nc.dram_tensor("wxa", (P, WXA), F16, kind="ExternalInput")
    wxb = nc.dram_tensor("wxb", (P, WXB), F16, kind="ExternalInput")
    outT = nc.dram_tensor("outT", (HD + 1, T), F16, kind="ExternalOutput")

    with TileContext(nc) as tc:
        with (
            tc.tile_pool(name="const", bufs=1) as const,
            tc.tile_pool(name="acts", bufs=1) as acts,
            tc.tile_pool(name="proj_ps", bufs=2, space="PSUM") as proj_ps,
            tc.tile_pool(name="tr_ps", bufs=1, space="PSUM") as tr_ps,
            tc.tile_pool(name="s_ps", bufs=2, space="PSUM") as s_ps,
            tc.tile_pool(name="o_ps", bufs=1, space="PSUM") as o_ps,
            tc.tile_pool(name="pwork", bufs=12) as pwork,
            tc.tile_pool(name="owork", bufs=2) as owork,
        ):
            # ---- PE warm-up first: throwaway matmuls keep the PE busy
            # through its clock-ramp window so real matmuls run at full
            # speed. Gated only on the wscr memset, not on any DMA. Any PE
            # idle gap resets the clock ramp, so the warmup count is sized
            # to bridge until the first x half lands.
            wscr = const.tile([P, CH], F16, name="wscr")
            nc.vector.memset(wscr[:], 0.0)
            for wu in range(NWARM):
                pswu = proj_ps.tile([P, CH], F32, name="warm", tag="proj")
                nc.tensor.matmul(pswu[:], wscr[:, 0:P], wscr[:],
                                 start=True, stop=True)

            # ---- input DMAs across THREE parallel DGE queues. Per-queue
            # transfers serialize and each dma_start costs ~3.4us fixed on
            # the first load (~1us after) + ~3us/MB, so everything qk0 needs
            # rides the FIRST load of each queue: combined [w1-half |
            # x0-half] tensors on sync and scalar. The later-needed
            # wv/biases ride the slower gpsimd SWDGE queue. Every transfer
            # is contiguous per partition. ----
            HB = DH * CH             # x half-chunk fp16 elems per partition
            wx_a = const.tile([P, WXA], F16, name="wx_a")
            nc.sync.dma_start(out=wx_a[:], in_=wxa[:, :])
            wx_b = const.tile([P, WXB], F16, name="wx_b")
            nc.scalar.dma_start(out=wx_b[:], in_=wxb[:, :])
            b_sb = wx_a[:, WXC:WXC + 4].bitcast(F32)  # [P, 2] f32 biases
            xq = {0: (wx_a, wx_b)}
            for ci in (1, 2, 3):
                ta = const.tile([P, HB], F16, name=f"xq{ci}a")
                tb = const.tile([P, HB], F16, name=f"xq{ci}b")
                base = ci * DT * CH
                nc.sync.dma_start(out=ta[:], in_=xp[:, base:base + HB])
                nc.scalar.dma_start(
                    out=tb[:], in_=xp[:, base + HB:base + DT * CH])
                xq[ci] = (ta, tb)

            def xqs(ci, d):
                parts = xq[ci]
                if len(parts) == 1:
                    return parts[0][:, d * CH:(d + 1) * CH]
                t = parts[0] if d < DH else parts[1]
                dd = d % DH
                off = DH * P if ci == 0 else 0
                return t[:, off + dd * CH:off + (dd + 1) * CH]

            ident = const.tile([P, P], F16, name="ident")
            make_identity(nc, ident)

            def w1s(d):
                t = wx_a if d < DH else wx_b
                dd = d % DH
                return t[:, dd * P:(dd + 1) * P]

            def wvs(d):
                return wx_b[:, WXC + d * HD:WXC + (d + 1) * HD]

            # ---- activations ----
            # qk1: rows 0-63 = Q^T, rows 64-127 = K^T; qk2: swapped halves.
            qk1 = acts.tile([P, T], F16, name="qk1")
            qk2 = acts.tile([P, T], F16, name="qk2")
            vT = acts.tile([HD, T], F16, name="vT")
            v_aug = acts.tile([P, NKT, HD + 1], F16, name="v_aug")
            nc.vector.memset(v_aug[:, :, HD], 1.0)

            def qk_chunk(ci):
                cs = slice(ci * CH, (ci + 1) * CH)
                ps = proj_ps.tile([P, CH], F32, name="proj", tag="proj")
                for d in range(DT):
                    nc.tensor.matmul(ps[:], w1s(d), xqs(ci, d),
                                     start=(d == 0), stop=(d == DT - 1))
                nc.vector.tensor_scalar_add(qk1[:, cs], ps[:], b_sb[:, 0:1])
                # half-swapped copy: qk2 = [K^T; Q^T]. 64-partition DVE ops
                # read any aligned src half and write either dest half.
                nc.vector.tensor_copy(qk2[0:HD, cs], qk1[HD:P, cs])
                nc.vector.tensor_copy(qk2[HD:P, cs], qk1[0:HD, cs])

            def v_mm(ca, cb, inter=()):
                # V projections for two chunks col-packed: chunk ca on array
                # columns 0-63, chunk cb on columns 64-127 -> the matmul pairs
                # overlap in the PE array; outputs land in disjoint halves of
                # one PSUM bank. `inter` maps d-index -> thunk emitted after
                # that d-step (scores pairs used as PE filler).
                psv = proj_ps.tile([P, CH], F32, name="projv", tag="proj")
                for d in range(DT):
                    nc.tensor.matmul(psv[0:HD, :], wvs(d), xqs(ca, d),
                                     start=(d == 0), stop=(d == DT - 1))
                    nc.tensor.matmul(psv[HD:P, :], wvs(d), xqs(cb, d),
                                     start=(d == 0), stop=(d == DT - 1))
                    if d in inter:
                        inter[d]()
                nc.vector.tensor_scalar_add(
                    vT[:, ca * CH:(ca + 1) * CH], psv[0:HD, :], b_sb[0:HD, 1:2])
                nc.vector.tensor_scalar_add(
                    vT[:, cb * CH:(cb + 1) * CH], psv[HD:P, :], b_sb[HD:P, 1:2])

            def v_tr(ca):
                for tt in range(4 * ca, 4 * ca + 8):
                    tp = tr_ps.tile([P, HD], F16, name="vtr", tag="vtr")
                    nc.tensor.transpose(tp[:], vT[:, tt * P:(tt + 1) * P],
                                        ident[:HD, :HD])
                    nc.vector.tensor_copy(v_aug[:, tt, 0:HD], tp[:])

            def chunk_pairs(ci):
                # diagonal pairs first (col-trimmed, masked), then full pairs
                return ([(4 * ci, 4 * ci + 1), (4 * ci + 2, 4 * ci + 3)]
                        + [(2 * j, 2 * j + 1) for j in range(2 * ci)])

            def scores_pair(ci, ka, kb, diag):
                c0 = ci * CH
                da = max(ka * P - c0, 0)  # first unmasked column
                db = max(kb * P - c0, 0)
                s2 = s_ps.tile([P, 2 * CH], F32, name="sT", tag="sT")
                # rows 0-63 of the array: K^T from qk2, Q^T from qk1
                nc.tensor.matmul(s2[:, da:CH],
                                 qk2[0:HD, ka * P:(ka + 1) * P],
                                 qk1[0:HD, c0 + da:c0 + CH],
                                 start=True, stop=True)
                # rows 64-127: K^T from qk1, Q^T from qk2 (concurrent)
                nc.tensor.matmul(s2[:, CH + db:2 * CH],
                                 qk1[HD:P, kb * P:(kb + 1) * P],
                                 qk2[HD:P, c0 + db:c0 + CH],
                                 start=True, stop=True)
                pt = pwork.tile([P, 2 * CH], F16, name="pT", tag="pT")
                if diag:
                    nc.scalar.activation(pt[:, da:CH], s2[:, da:CH],
                                         mybir.ActivationFunctionType.Exp)
                    nc.scalar.activation(pt[:, CH + db:2 * CH],
                                         s2[:, CH + db:2 * CH],
                                         mybir.ActivationFunctionType.Exp)
                    # causal mask on the 128-col diagonal band only:
                    # keep where (query - delta) >= key  <=>  c' >= p
                    for off in (da, CH + db):
                        nc.gpsimd.affine_select(
                            out=pt[:, off:off + P],
                            in_=pt[:, off:off + P],
                            compare_op=mybir.AluOpType.is_ge, fill=0.0,
                            base=0, pattern=[[1, P]],
                            channel_multiplier=-1,
                        )
                else:
                    nc.scalar.activation(pt[:], s2[:],
                                         mybir.ActivationFunctionType.Exp)
                return pt

            def pv_pair(ci, ops, ka, kb, pt, first, last):
                c0 = ci * CH
                da = max(ka * P - c0, 0)
                db = max(kb * P - c0, 0)
                nc.tensor.matmul(ops[:, da:CH], v_aug[:, ka, :],
                                 pt[:, da:CH],
                                 start=first, stop=False)
                nc.tensor.matmul(ops[:, db:CH], v_aug[:, kb, :],
                                 pt[:, CH + db:2 * CH],
                                 start=False, stop=last)

            def store_chunk(ci, ops):
                # f16 output (error budget ~1e-3 << 2e-2 gate) halves the
                # store transfers; the host divides in f32.
                osb = owork.tile([HD + 1, CH], F16, name="osb", tag="osb")
                if ci == NCH - 1:
                    # final store: ACT is idle once the last EXP retires, so
                    # the PSUM->SBUF copy splits across ACT+DVE in parallel
                    nc.scalar.copy(osb[:, 0:CH // 2], ops[:, 0:CH // 2])
                    nc.vector.tensor_copy(osb[:, CH // 2:CH],
                                          ops[:, CH // 2:CH])
                else:
                    nc.vector.tensor_copy(osb[:], ops[:])
                nc.sync.dma_start(
                    out=outT[:, ci * CH:(ci + 1) * CH], in_=osb[:])

            # ---- global software pipeline ----
            # All four QK projections run as early as their x halves land;
            # scores stream chunk-major so the ACT engine (EXP) is fed
            # continuously from first score to last; PV lags behind its
            # chunk's scores, and the V-projection / transpose blocks act
            # as PE filler between score pairs (each scores pair waits on
            # an s_ps bank freed at EXP rate, so heavy PE work is
            # interleaved between them to avoid head-of-line stalls).
            pts = {}
            opses = {}

            def sc(ci, j):
                ka, kb = chunk_pairs(ci)[j]
                pts[(ci, j)] = scores_pair(ci, ka, kb, diag=j < 2)

            def pv(ci, j):
                pairs = chunk_pairs(ci)
                ka, kb = pairs[j]
                pv_pair(ci, opses[ci], ka, kb, pts.pop((ci, j)),
                        first=j == 0, last=j == len(pairs) - 1)

            qk_chunk(0)
            sc(0, 0); sc(0, 1)
            qk_chunk(1)
            sc(1, 0); sc(1, 1)
            qk_chunk(2)
            sc(1, 2); sc(1, 3)
            v_mm(0, 1, inter={2: lambda: sc(2, 0), 5: lambda: sc(2, 1)})
            sc(2, 2)
            qk_chunk(3)
            sc(2, 3)
            v_tr(0)
            sc(2, 4)
            opses[0] = o_ps.tile([HD + 1, CH], F32, name="oacc", tag="oacc")
            pv(0, 0); pv(0, 1)
            sc(2, 5)
            store_chunk(0, opses[0])
            opses[1] = o_ps.tile([HD + 1, CH], F32, name="oacc", tag="oacc")
            pv(1, 0); pv(1, 1)
            sc(3, 0)
            pv(1, 2); pv(1, 3)
            sc(3, 1)
            store_chunk(1, opses[1])
            v_mm(2, 3, inter={3: lambda: sc(3, 2), 6: lambda: sc(3, 3)})
            v_tr(2)
            sc(3, 4)
            # chunk 3's accumulator lives in the transpose pool's PSUM bank
            # (free after v_tr(2)) so pv3 can interleave with pv2 instead of
            # serializing after store2's copy frees the o_ps bank.
            opses[2] = o_ps.tile([HD + 1, CH], F32, name="oacc", tag="oacc")
            opses[3] = tr_ps.tile([HD + 1, CH], F32, name="oacc3", tag="vtr")
            pv(2, 0); pv(2, 1)
            sc(3, 5)
            pv(3, 0)
            pv(2, 2); pv(2, 3)
            sc(3, 6)
            pv(3, 1)
            pv(2, 4)
            sc(3, 7)
            pv(3, 2)
            pv(2, 5)
            store_chunk(2, opses[2])
            for j in range(3, 8):
                pv(3, j)
            store_chunk(3, opses[3])

    if legalize:
        _legalize_waits(nc, mybir)
    return nc


def _legalize_waits(nc, mybir):
    """Split multi-wait instructions for the XLA-route walrus codegen.

    The TPB EVENTS struct holds one semaphore wait per instruction and this
    pipeline's codegen refuses >1. Hoist extra waits onto standalone
    EventSemaphore instructions on the same engine queue right before the
    instruction - semantically identical, the queue stalls there.
    """
    n = 0
    for f in nc.m.functions:
        for b in f.blocks:
            out = []
            changed = False
            for inst in b.instructions:
                si = inst.sync_info
                waits = list(si.on_wait) if si is not None and si.on_wait else []
                if len(waits) > 1:
                    changed = True
                    for w in waits[:-1]:
                        n += 1
                        out.append(mybir.InstEventSemaphore(
                            name=f"waitfix{n}_{inst.name}",
                            engine=inst.engine,
                            sync_info=mybir.SyncInfo(on_wait=[w], on_update=[]),
                        ))
                    inst.sync_info = mybir.SyncInfo(
                        on_wait=waits[-1:],
                        on_update=list(si.on_update or []),
                    )
                out.append(inst)
            if changed:
                b.instructions = out
    return n


def kernel(x, wq, bq, wk, bk, wv, bv):
    global LAST_RESULTS
    import os
    os.environ.setdefault("JAX_PLATFORMS", "")
    from concourse.bass_utils import run_bass_kernel_spmd

    x = np.asarray(x, dtype=np.float32)
    s = np.float32(1.0 / np.sqrt(HD))
    # per partition p (= row of the D-contraction tile), d-major columns
    w1 = np.concatenate([np.asarray(wq, np.float32) * s,
                         np.asarray(wk, np.float32)], 0).T  # (D, 128)
    w1d = np.ascontiguousarray(
        w1.reshape(DT, P, P).transpose(1, 0, 2)
        .reshape(P, DT * P)).astype(np.float16)
    wv_t = np.asarray(wv, np.float32).T                      # (D, 64)
    wvd = np.ascontiguousarray(
        wv_t.reshape(DT, P, HD).transpose(1, 0, 2)
        .reshape(P, DT * HD)).astype(np.float16)
    b1 = np.concatenate([np.asarray(bq, np.float32) * s,
                         np.asarray(bk, np.float32)])
    bv_f = np.asarray(bv, np.float32)
    bb = np.ascontiguousarray(
        np.stack([b1, np.concatenate([bv_f, bv_f])], axis=1))  # (P, 2)
    # xp[b]: partition-major, chunk-major, d-major: row p holds, for each
    # chunk ci and d-tile d, the 512 fp16 values x[b, ci*CH:(ci+1)*CH, d*P+p].
    xp = np.ascontiguousarray(
        x.reshape(B, NCH, CH, DT, P).transpose(0, 4, 1, 3, 2)
        .reshape(B, P, NCH * DT * CH)).astype(np.float16)
    # combined first loads: [w1 d-half | x0 d-half | biases or wv] per queue
    DH = DT // 2
    bbf16 = bb.astype(np.float32).view(np.float16)  # (P, 4) raw bias bytes
    wxa_b = [np.ascontiguousarray(np.concatenate(
        [w1d[:, :DH * P], xp[b, :, :DH * CH], bbf16], axis=1))
        for b in range(B)]
    wxb_b = [np.ascontiguousarray(np.concatenate(
        [w1d[:, DH * P:], xp[b, :, DH * CH:DT * CH], wvd], axis=1))
        for b in range(B)]

    nc = _build_module()
    in_maps = [
        {"xp": xp[b], "wxa": wxa_b[b], "wxb": wxb_b[b]}
        for b in range(B)
    ]
    res = None
    for attempt in range(3):
        try:
            res = run_bass_kernel_spmd(nc, in_maps, core_ids=list(range(B)))
            break
        except Exception:
            # transient device wedges (NRT_EXEC_UNIT_UNRECOVERABLE) happen;
            # rebuild the module and retry on a clean execution
            if attempt == 2:
                raise
            nc = _build_module()
    LAST_RESULTS = res

    out = np.empty((B, T, HD), dtype=np.float32)
    for b in range(B):
        # (65, T) f16: rows 0..63 = O^T, row 64 = denom; divide in f32
        oT = np.asarray(res.results[b]["outT"], dtype=np.float32)
        out[b] = (oT[:HD] / oT[HD:HD + 1]).T
    return out


# revision 47
# speedup vs baseline: 1.0452x; 1.0452x over previous
"""Trainium2 Bass kernel for a single causal attention head.

Problem: x:(8,2048,1024) f32, per-head projections wq/wk/wv:(64,1024),
biases (64,). Output: softmax(causal(q k^T / sqrt(64))) @ v : (8,2048,64).

Strategy:
  - Data-parallel: batch b -> core b (8 cores, 1 batch each).
  - Host prep packs every input into partition-major, fully contiguous
    per-partition lines so each DMA is ~128 large descriptors:
      * xp:(P, NCH*DT*CH) fp16 - x[b] chunk-major/d-major per partition
        (8KB contiguous per partition per chunk).
      * wall:(P, DT*(P+HD)) fp16 - [wq*s|wk] and wv interleaved per d-tile.
      * bb:(P, 2) f32 - [bq*s;bk] and [bv;bv] columns.
  - Device (per core):
      * qk1 = [wq|wk]^T.T @ x: rows 0-63 = Q^T, rows 64-127 = K^T (PSUM
        accumulate over 8 d-tiles, fp16 matmuls, N=512 chunks).
      * qk2 = half-swapped copy of qk1 -> both Q^T and K^T live on both
        partition halves; scores for two k-tiles share the PE array via
        row packing.
      * vT (64,T) fp16, transposed back to (T,64) tiles via fp16 PE
        transpose, augmented with a ones column (softmax denominator
        rides along the PV matmul).
      * S^T = K^T.T @ Q^T per k-tile; P^T = exp(S^T) on ACT; causal mask
        via gpsimd affine_select restricted to the 128-col diagonal band.
      * Diagonal pairs run FIRST per chunk with column-trimmed scores/
        exp/mask/PV (fully-masked columns never computed); non-diagonal
        pairs follow full-range.
      * O^T_aug[65, T] accumulated in PSUM over k-tiles; row 64 = sum_j P^T.
      * attention for chunk ci emitted right after projection chunk ci.
  - Host post: out[b] = (O^T[0:64] / O^T[64:65]).T  (softmax normalization).
"""

import numpy as np

B, T, D, HD = 8, 2048, 1024, 64
P = 128          # SBUF partitions
CH = 512         # q-chunk (matmul moving dim)
NCH = T // CH    # 4
DT = D // P      # 8 d-tiles
NKT = T // P     # 16 k-tiles
NWARM = 13       # PE clock-ramp warmup matmuls
DH = DT // 2     # d-tiles per combined/half x load (two DMA queues)

LAST_RESULTS = None      # BassKernelResults of the most recent run (for test.py)


def _build_module(legalize=True):
    import concourse.bass as bass
    import concourse.mybir as mybir
    from concourse.tile import TileContext

    from concourse.masks import make_identity
    F32 = mybir.dt.float32
    F16 = mybir.dt.float16

    nc = bass.Bass("TRN2", target_bir_lowering=True)

    WXC = DH * (P + CH)  # cols of a combined [w1-half | x0-half] tensor
    WXA = WXC + 4            # wxa also carries the biases (4 f16 = 2 f32)
    WXB = WXC + DT * HD      # wxb also carries wv
    xp = nc.dram_tensor("xp", (P, NCH * DT * CH), F16, kind="ExternalInput")
    wxa = nc.dram_tensor("wxa", (P, WXA), F16, kind="ExternalInput")
    wxb = nc.dram_tensor("wxb", (P, WXB), F16, kind="ExternalInput")
    outT = nc.dram_tensor("outT", (HD + 1, T), F16, kind="ExternalOutput")

    with TileContext(nc) as tc:
        with (
            tc.tile_pool(name="const", bufs=1) as const,
            tc.tile_pool(name="acts", bufs=1) as acts,
            tc.tile_pool(name="proj_ps", bufs=2, space="PSUM") as proj_ps,
            tc.tile_pool(name="tr_ps", bufs=1, space="PSUM") as tr_ps,
            tc.tile_pool(name="s_ps", bufs=2, space="PSUM") as s_ps,
            tc.tile_pool(name="o_ps", bufs=1, space="PSUM") as o_ps,
            tc.tile_pool(name="pwork", bufs=12) as pwork,
            tc.tile_pool(name="owork", bufs=3) as owork,
        ):
            # ---- PE warm-up first: throwaway matmuls keep the PE busy
            # through its clock-ramp window so real matmuls run at full
            # speed. Gated only on the wscr memset, not on any DMA. Any PE
            # idle gap resets the clock ramp, so the warmup count is sized
            # to bridge until the first x half lands.
            wscr = const.tile([P, CH], F16, name="wscr")
            nc.vector.memset(wscr[:], 0.0)
            for wu in range(NWARM):
                pswu = proj_ps.tile([P, CH], F32, name="warm", tag="proj")
                nc.tensor.matmul(pswu[:], wscr[:, 0:P], wscr[:],
                                 start=True, stop=True)

            # ---- input DMAs across THREE parallel DGE queues. Per-queue
            # transfers serialize and each dma_start costs ~3.4us fixed on
            # the first load (~1us after) + ~3us/MB, so everything qk0 needs
            # rides the FIRST load of each queue: combined [w1-half |
            # x0-half] tensors on sync and scalar. The later-needed
            # wv/biases ride the slower gpsimd SWDGE queue. Every transfer
            # is contiguous per partition. ----
            HB = DH * CH             # x half-chunk fp16 elems per partition
            wx_a = const.tile([P, WXA], F16, name="wx_a")
            nc.sync.dma_start(out=wx_a[:], in_=wxa[:, :])
            wx_b = const.tile([P, WXB], F16, name="wx_b")
            nc.scalar.dma_start(out=wx_b[:], in_=wxb[:, :])
            b_sb = wx_a[:, WXC:WXC + 4].bitcast(F32)  # [P, 2] f32 biases
            xq = {0: (wx_a, wx_b)}
            for ci in (1, 2, 3):
                ta = const.tile([P, HB], F16, name=f"xq{ci}a")
                tb = const.tile([P, HB], F16, name=f"xq{ci}b")
                base = ci * DT * CH
                nc.sync.dma_start(out=ta[:], in_=xp[:, base:base + HB])
                nc.scalar.dma_start(
                    out=tb[:], in_=xp[:, base + HB:base + DT * CH])
                xq[ci] = (ta, tb)

            def xqs(ci, d):
                parts = xq[ci]
                if len(parts) == 1:
                    return parts[0][:, d * CH:(d + 1) * CH]
                t = parts[0] if d < DH else parts[1]
                dd = d % DH
                off = DH * P if ci == 0 else 0
                return t[:, off + dd * CH:off + (dd + 1) * CH]

            ident = const.tile([P, P], F16, name="ident")
            make_identity(nc, ident)

            def w1s(d):
                t = wx_a if d < DH else wx_b
                dd = d % DH
                return t[:, dd * P:(dd + 1) * P]

            def wvs(d):
                return wx_b[:, WXC + d * HD:WXC + (d + 1) * HD]

            # ---- activations ----
            # qk1: rows 0-63 = Q^T, rows 64-127 = K^T; qk2: swapped halves.
            qk1 = acts.tile([P, T], F16, name="qk1")
            qk2 = acts.tile([P, T], F16, name="qk2")
            vT = acts.tile([HD, T], F16, name="vT")
            v_aug = acts.tile([P, NKT, HD + 1], F16, name="v_aug")
            nc.vector.memset(v_aug[:, :, HD], 1.0)

            def qk_chunk(ci):
                cs = slice(ci * CH, (ci + 1) * CH)
                ps = proj_ps.tile([P, CH], F32, name="proj", tag="proj")
                for d in range(DT):
                    nc.tensor.matmul(ps[:], w1s(d), xqs(ci, d),
                                     start=(d == 0), stop=(d == DT - 1))
                nc.vector.tensor_scalar_add(qk1[:, cs], ps[:], b_sb[:, 0:1])
                # half-swapped copy: qk2 = [K^T; Q^T]. 64-partition DVE ops
                # read any aligned src half and write either dest half.
                nc.vector.tensor_copy(qk2[0:HD, cs], qk1[HD:P, cs])
                nc.vector.tensor_copy(qk2[HD:P, cs], qk1[0:HD, cs])

            def v_mm(ca, cb, inter=()):
                # V projections for two chunks col-packed: chunk ca on array
                # columns 0-63, chunk cb on columns 64-127 -> the matmul pairs
                # overlap in the PE array; outputs land in disjoint halves of
                # one PSUM bank. `inter` maps d-index -> thunk emitted after
                # that d-step (scores pairs used as PE filler).
                psv = proj_ps.tile([P, CH], F32, name="projv", tag="proj")
                for d in range(DT):
                    nc.tensor.matmul(psv[0:HD, :], wvs(d), xqs(ca, d),
                                     start=(d == 0), stop=(d == DT - 1))
                    nc.tensor.matmul(psv[HD:P, :], wvs(d), xqs(cb, d),
                                     start=(d == 0), stop=(d == DT - 1))
                    if d in inter:
                        inter[d]()
                nc.vector.tensor_scalar_add(
                    vT[:, ca * CH:(ca + 1) * CH], psv[0:HD, :], b_sb[0:HD, 1:2])
                nc.vector.tensor_scalar_add(
                    vT[:, cb * CH:(cb + 1) * CH], psv[HD:P, :], b_sb[HD:P, 1:2])

            def v_tr(ca):
                for tt in range(4 * ca, 4 * ca + 8):
                    tp = tr_ps.tile([P, HD], F16, name="vtr", tag="vtr")
                    nc.tensor.transpose(tp[:], vT[:, tt * P:(tt + 1) * P],
                                        ident[:HD, :HD])
                    nc.vector.tensor_copy(v_aug[:, tt, 0:HD], tp[:])

            def chunk_pairs(ci):
                # diagonal pairs first (col-trimmed, masked), then full pairs
                return ([(4 * ci, 4 * ci + 1), (4 * ci + 2, 4 * ci + 3)]
                        + [(2 * j, 2 * j + 1) for j in range(2 * ci)])

            def scores_pair(ci, ka, kb, diag):
                c0 = ci * CH
                da = max(ka * P - c0, 0)  # first unmasked column
                db = max(kb * P - c0, 0)
                s2 = s_ps.tile([P, 2 * CH], F32, name="sT", tag="sT")
                # rows 0-63 of the array: K^T from qk2, Q^T from qk1
                nc.tensor.matmul(s2[:, da:CH],
                                 qk2[0:HD, ka * P:(ka + 1) * P],
                                 qk1[0:HD, c0 + da:c0 + CH],
                                 start=True, stop=True)
                # rows 64-127: K^T from qk1, Q^T from qk2 (concurrent)
                nc.tensor.matmul(s2[:, CH + db:2 * CH],
                                 qk1[HD:P, kb * P:(kb + 1) * P],
                                 qk2[HD:P, c0 + db:c0 + CH],
                                 start=True, stop=True)
                pt = pwork.tile([P, 2 * CH], F16, name="pT", tag="pT")
                if diag:
                    nc.scalar.activation(pt[:, da:CH], s2[:, da:CH],
                                         mybir.ActivationFunctionType.Exp)
                    nc.scalar.activation(pt[:, CH + db:2 * CH],
                                         s2[:, CH + db:2 * CH],
                                         mybir.ActivationFunctionType.Exp)
                    # causal mask on the 128-col diagonal band only:
                    # keep where (query - delta) >= key  <=>  c' >= p
                    for off in (da, CH + db):
                        nc.gpsimd.affine_select(
                            out=pt[:, off:off + P],
                            in_=pt[:, off:off + P],
                            compare_op=mybir.AluOpType.is_ge, fill=0.0,
                            base=0, pattern=[[1, P]],
                            channel_multiplier=-1,
                        )
                else:
                    nc.scalar.activation(pt[:], s2[:],
                                         mybir.ActivationFunctionType.Exp)
                return pt

            def pv_pair(ci, ops, ka, kb, pt, first, last):
                c0 = ci * CH
                da = max(ka * P - c0, 0)
                db = max(kb * P - c0, 0)
                nc.tensor.matmul(ops[:, da:CH], v_aug[:, ka, :],
                                 pt[:, da:CH],
                                 start=first, stop=False)
                nc.tensor.matmul(ops[:, db:CH], v_aug[:, kb, :],
                                 pt[:, CH + db:2 * CH],
                                 start=False, stop=last)

            def store_chunk(ci, ops):
                # f16 output (error budget ~1e-3 << 2e-2 gate) halves the
                # store transfers; the host divides in f32.
                osb = owork.tile([HD + 1, CH], F16, name="osb", tag="osb")
                nc.vector.tensor_copy(osb[:], ops[:])
                nc.sync.dma_start(
                    out=outT[:, ci * CH:(ci + 1) * CH], in_=osb[:])

            # ---- global software pipeline ----
            # All four QK projections run as early as their x halves land;
            # scores stream chunk-major so the ACT engine (EXP) is fed
            # continuously from first score to last; PV lags behind its
            # chunk's scores, and the V-projection / transpose blocks act
            # as PE filler between score pairs (each scores pair waits on
            # an s_ps bank freed at EXP rate, so heavy PE work is
            # interleaved between them to avoid head-of-line stalls).
            pts = {}
            opses = {}

            def sc(ci, j):
                ka, kb = chunk_pairs(ci)[j]
                pts[(ci, j)] = scores_pair(ci, ka, kb, diag=j < 2)

            def pv(ci, j):
                pairs = chunk_pairs(ci)
                ka, kb = pairs[j]
                if ci == 3:
                    pv_pair3(ka, kb, pts.pop((ci, j)), j,
                             last=j == len(pairs) - 1)
                else:
                    pv_pair(ci, opses[ci], ka, kb, pts.pop((ci, j)),
                            first=j == 0, last=j == len(pairs) - 1)

            M = CH // 2

            def pv_pair3(ka, kb, pt, j, last):
                # chunk 3's accumulator is split into column halves living
                # in two PSUM banks, so the a-half's copy+store overlap the
                # b-half's final PV matmuls instead of serializing after
                # them; the two stores ride different DMA queues.
                c0 = 3 * CH
                da = max(ka * P - c0, 0)
                db = max(kb * P - c0, 0)
                oa, ob = opses[3]
                if da < M:
                    nc.tensor.matmul(oa[:, da:M], v_aug[:, ka, :],
                                     pt[:, da:M],
                                     start=(j == 0), stop=False)
                if db < M:
                    nc.tensor.matmul(oa[:, db:M], v_aug[:, kb, :],
                                     pt[:, CH + db:CH + M],
                                     start=False, stop=last)
                if last:
                    osb_a = owork.tile([HD + 1, M], F16, name="osb3a",
                                       tag="osb")
                    nc.vector.tensor_copy(osb_a[:], oa[:])
                    nc.scalar.dma_start(out=outT[:, c0:c0 + M],
                                        in_=osb_a[:])
                ba = max(da, M)
                bb2 = max(db, M)
                nc.tensor.matmul(ob[:, ba - M:M], v_aug[:, ka, :],
                                 pt[:, ba:CH],
                                 start=(j == 0), stop=False)
                nc.tensor.matmul(ob[:, bb2 - M:M], v_aug[:, kb, :],
                                 pt[:, CH + bb2:2 * CH],
                                 start=False, stop=last)
                if last:
                    osb_b = owork.tile([HD + 1, M], F16, name="osb3b",
                                       tag="osb")
                    nc.vector.tensor_copy(osb_b[:], ob[:])
                    nc.sync.dma_start(out=outT[:, c0 + M:c0 + 2 * M],
                                      in_=osb_b[:])

            qk_chunk(0)
            sc(0, 0); sc(0, 1)
            qk_chunk(1)
            sc(1, 0); sc(1, 1)
            qk_chunk(2)
            sc(1, 2); sc(1, 3)
            v_mm(0, 1, inter={2: lambda: sc(2, 0), 5: lambda: sc(2, 1)})
            sc(2, 2)
            qk_chunk(3)
            sc(2, 3)
            v_tr(0)
            sc(2, 4)
            opses[0] = o_ps.tile([HD + 1, CH], F32, name="oacc", tag="oacc")
            pv(0, 0); pv(0, 1)
            sc(2, 5)
            store_chunk(0, opses[0])
            opses[1] = o_ps.tile([HD + 1, CH], F32, name="oacc", tag="oacc")
            pv(1, 0); pv(1, 1)
            sc(3, 0)
            pv(1, 2); pv(1, 3)
            sc(3, 1)
            store_chunk(1, opses[1])
            v_mm(2, 3, inter={3: lambda: sc(3, 2), 6: lambda: sc(3, 3)})
            v_tr(2)
            sc(3, 4)
            # chunk 3's accumulator lives in the transpose pool's PSUM bank
            # (free after v_tr(2)) so pv3 can interleave with pv2 instead of
            # serializing after store2's copy frees the o_ps bank.
            opses[2] = o_ps.tile([HD + 1, CH], F32, name="oacc", tag="oacc")
            opses[3] = (
                tr_ps.tile([HD + 1, M], F32, name="oacc3a", tag="vtr"),
                proj_ps.tile([HD + 1, M], F32, name="oacc3b", tag="proj"),
            )
            pv(2, 0); pv(2, 1)
            sc(3, 5)
            pv(3, 0)
            pv(2, 2); pv(2, 3)
            sc(3, 6)
            pv(3, 1)
            pv(2, 4)
            sc(3, 7)
            pv(3, 2)
            pv(2, 5)
            store_chunk(2, opses[2])
            for j in range(3, 8):
                pv(3, j)

    if legalize:
        _legalize_waits(nc, mybir)
    return nc


def _legalize_waits(nc, mybir):
    """Split multi-wait instructions for the XLA-route walrus codegen.

    The TPB EVENTS struct holds one semaphore wait per instruction and this
    pipeline's codegen refuses >1. Hoist extra waits onto standalone
    EventSemaphore instructions on the same engine queue right before the
    instruction - semantically identical, the queue stalls there.
    """
    n = 0
    for f in nc.m.functions:
        for b in f.blocks:
            out = []
            changed = False
            for inst in b.instructions:
                si = inst.sync_info
                waits = list(si.on_wait) if si is not None and si.on_wait else []
                if len(waits) > 1:
                    changed = True
                    for w in waits[:-1]:
                        n += 1
                        out.append(mybir.InstEventSemaphore(
                            name=f"waitfix{n}_{inst.name}",
                            engine=inst.engine,
                            sync_info=mybir.SyncInfo(on_wait=[w], on_update=[]),
                        ))
                    inst.sync_info = mybir.SyncInfo(
                        on_wait=waits[-1:],
                        on_update=list(si.on_update or []),
                    )
                out.append(inst)
            if changed:
                b.instructions = out
    return n


def kernel(x, wq, bq, wk, bk, wv, bv):
    global LAST_RESULTS
    import os
    os.environ.setdefault("JAX_PLATFORMS", "")
    from concourse.bass_utils import run_bass_kernel_spmd

    x = np.asarray(x, dtype=np.float32)
    s = np.float32(1.0 / np.sqrt(HD))
    # per partition p (= row of the D-contraction tile), d-major columns
    w1 = np.concatenate([np.asarray(wq, np.float32) * s,
                         np.asarray(wk, np.float32)], 0).T  # (D, 128)
    w1d = np.ascontiguousarray(
        w1.reshape(DT, P, P).transpose(1, 0, 2)
        .reshape(P, DT * P)).astype(np.float16)
    wv_t = np.asarray(wv, np.float32).T                      # (D, 64)
    wvd = np.ascontiguousarray(
        wv_t.reshape(DT, P, HD).transpose(1, 0, 2)
        .reshape(P, DT * HD)).astype(np.float16)
    b1 = np.concatenate([np.asarray(bq, np.float32) * s,
                         np.asarray(bk, np.float32)])
    bv_f = np.asarray(bv, np.float32)
    bb = np.ascontiguousarray(
        np.stack([b1, np.concatenate([bv_f, bv_f])], axis=1))  # (P, 2)
    # xp[b]: partition-major, chunk-major, d-major: row p holds, for each
    # chunk ci and d-tile d, the 512 fp16 values x[b, ci*CH:(ci+1)*CH, d*P+p].
    xp = np.ascontiguousarray(
        x.reshape(B, NCH, CH, DT, P).transpose(0, 4, 1, 3, 2)
        .reshape(B, P, NCH * DT * CH)).astype(np.float16)
    # combined first loads: [w1 d-half | x0 d-half | biases or wv] per queue
    DH = DT // 2
    bbf16 = bb.astype(np.float32).view(np.float16)  # (P, 4) raw bias bytes
    wxa_b = [np.ascontiguousarray(np.concatenate(
        [w1d[:, :DH * P], xp[b, :, :DH * CH], bbf16], axis=1))
        for b in range(B)]
    wxb_b = [np.ascontiguousarray(np.concatenate(
        [w1d[:, DH * P:], xp[b, :, DH * CH:DT * CH], wvd], axis=1))
        for b in range(B)]

    nc = _build_module()
    in_maps = [
        {"xp": xp[b], "wxa": wxa_b[b], "wxb": wxb_b[b]}
        for b in range(B)
    ]
    res = None
    for attempt in range(3):
        try:
            res = run_bass_kernel_spmd(nc, in_maps, core_ids=list(range(B)))
            break
        except Exception:
            # transient device wedges (NRT_EXEC_UNIT_UNRECOVERABLE) happen;
            # rebuild the module and retry on a clean execution
            if attempt == 2:
                raise
            nc = _build_module()
    LAST_RESULTS = res

    out = np.empty((B, T, HD), dtype=np.float32)
    for b in range(B):
        # (65, T) f16: rows 0..63 = O^T, row 64 = denom; divide in f32
        oT = np.asarray(res.results[b]["outT"], dtype=np.float32)
        out[b] = (oT[:HD] / oT[HD:HD + 1]).T
    return out
